# revision 16
# baseline (speedup 1.0000x reference)
"""Trainium2 Bass kernel for a 3x3 VALID conv: x[64,256,256] * k[128,64,3,3] -> [128,254,254].

Strategy:
  - Shard output rows across 8 cores (32 rows each; 8*32=256 >= 254, tail padded).
  - Per core, conv is 6 accumulated matmuls per pair of output rows:
      contraction K=128 = 64 in-channels x 2 kernel rows (kh=0,1 packed in the
      partition dim via a row-shifted duplicate of x on partitions 64..127);
      kh=2 runs as 3 more K=128 matmuls whose lower-half weights are zero.
    M=128 output channels, N=508 = 2 output rows x 254 cols (one PSUM bank).
  - PSUM evacuation fused with the bias add on the Vector engine.
  - Host gathers the 8 per-core output slabs.
"""

import os
import sys

import numpy as np

for _p in ("/opt/trn_rl_repo", "/root/.axon_site/_ro/trn_rl_repo"):
    if os.path.isdir(_p) and _p not in sys.path:
        sys.path.insert(0, _p)

from concourse import bass, mybir, tile  # noqa: E402
from concourse.bass_utils import run_bass_kernel_spmd  # noqa: E402

IN_C, H, W = 64, 256, 256
KS = 3
OUT_C = 128
OH, OW = H - KS + 1, W - KS + 1  # 254, 254
N_CORES = 8
RPC = 32          # output rows computed per core (8*32 = 256 >= 254)
PAD_H = 259       # padded input rows so core 7 can read h0+34 = 258

# x is staged per 4-pair chunk (8 output rows) in separate SBUF tiles so loads
# overlap compute at fine granularity. Chunk c needs q-rows [8c, 8c+10) -> 10
# rows incl. 2-row halo.
N_CHUNKS = 4
PAIRS_PER_CHUNK = 4
QC = 10           # q-rows per chunk tile

# Matmul dtype: "f32r" (full-rate fp32-ish), "bf16", or "f32" (exact, 4x slower)
MM_DT = os.environ.get("CONV_MM_DT", "f32r")

TRACE = False
LAST_RESULTS = None

_COMPILED = {}


def _np_dt(mm_dt):
    if mm_dt == "bf16":
        import ml_dtypes

        return np.dtype(ml_dtypes.bfloat16)
    return np.dtype(np.float32)


def _bass_dt(mm_dt):
    return {
        "bf16": mybir.dt.bfloat16,
        "f32r": mybir.dt.float32r,
        "f32": mybir.dt.float32,
    }[mm_dt]


def _build_program(mm_dt):
    dt = _bass_dt(mm_dt)
    f32 = mybir.dt.float32
    nc = bass.Bass()

    x_ext = nc.declare_dram_parameter(
        "xdup", [128, N_CHUNKS * QC * W], dt, isOutput=False
    )
    w_ext = nc.declare_dram_parameter("wpack", [128, 6 * 128], dt, isOutput=False)
    b_ext = nc.declare_dram_parameter("bias", [128, 1], f32, isOutput=False)
    o_ext = nc.declare_dram_parameter("out", [128, RPC * OW], f32, isOutput=True)

    with tile.TileContext(nc) as tc:
        with (
            tc.tile_pool(name="wpool", bufs=1) as wpool,
            # bufs = N_CHUNKS: chunk tiles are never reused -> no WAW/WAR
            # waits on the loads.
            tc.tile_pool(name="xpool", bufs=N_CHUNKS) as xpool,
            tc.tile_pool(name="pspool", bufs=4, space="PSUM") as pspool,
            # bufs = n pairs: output tiles are never reused -> evacuations
            # only ever wait on their PSUM producer.
            tc.tile_pool(name="opool", bufs=N_CHUNKS * PAIRS_PER_CHUNK) as opool,
        ):
            # Loads dispatch from the ACT HWDGE sequencer, stores from SP:
            # a store's hoisted DVE wait then never stalls a load dispatch.
            wt = wpool.tile([128, 6 * 128], dt)
            nc.scalar.dma_start(out=wt[:], in_=w_ext[:])
            bt = wpool.tile([128, 1], f32)
            nc.scalar.dma_start(out=bt[:], in_=b_ext[:])

            wv = wt[:].rearrange("p (s m) -> p s m", m=128)
            ov = o_ext.rearrange("p (r w) -> p r w", w=OW)

            for c in range(N_CHUNKS):
                xt = xpool.tile([128, QC * W], dt)
                nc.scalar.dma_start(
                    out=xt[:], in_=x_ext[:, c * QC * W : (c + 1) * QC * W]
                )
                xv = xt[:].rearrange("p (q w) -> p q w", w=W)
                for lp in range(PAIRS_PER_CHUNK):
                    r = 2 * lp  # chunk-local output row
                    ps = pspool.tile([128, 2 * OW], f32)
                    for j in range(6):
                        kw = j % 3
                        q0 = r if j < 3 else r + 2
                        nc.tensor.matmul(
                            ps[:],
                            lhsT=wv[:, j, :],
                            rhs=xv[:, q0 : q0 + 2, kw : kw + OW],
                            start=(j == 0),
                            stop=(j == 5),
                        )
                    so = opool.tile([128, 2 * OW], f32)
                    nc.vector.tensor_scalar_add(so[:], ps[:], bt[:, 0:1])
                    rg = 2 * PAIRS_PER_CHUNK * c + r
                    nc.sync.dma_start(out=ov[:, rg : rg + 2, :], in_=so[:])

    _split_multi_waits(nc)
    return nc


def _split_multi_waits(nc):
    """Walrus codegen accepts a single sync-wait command per instruction.

    Tile's sem assignment happily attaches several. Hoist all but the last
    wait of every instruction onto fresh NoOps placed immediately before it
    on the same engine stream (engine streams execute in program order, so
    semantics are preserved; the wait merely moves from the instruction to
    its dispatching sequencer).
    """
    for fn in nc.m.functions:
        for bb in fn.blocks:
            out = []
            for inst in bb.instructions:
                si = inst.sync_info
                waits = list(si.on_wait) if si is not None and si.on_wait else []
                if len(waits) > 1:
                    for wt_ in waits[:-1]:
                        nop = mybir.InstNoOp(
                            name=nc.get_next_instruction_name(),
                            engine=inst.engine,
                        )
                        nop.sync_info = mybir.SyncInfo(
                            on_wait=[wt_], on_update=[]
                        )
                        nc.register_instruction(nop)
                        out.append(nop)
                    inst.sync_info = mybir.SyncInfo(
                        on_wait=[waits[-1]], on_update=list(si.on_update)
                    )
                out.append(inst)
            bb.instructions = out


def _get_program(mm_dt):
    if mm_dt not in _COMPILED:
        _COMPILED[mm_dt] = _build_program(mm_dt)
    return _COMPILED[mm_dt]


def _prep_inputs(x, kernels, biases, mm_dt):
    np_dt = _np_dt(mm_dt)
    xp = np.zeros((IN_C, PAD_H, W), dtype=np.float32)
    xp[:, :H] = x
    xp = xp.astype(np_dt)

    # wpack[:, s, :]: s=kw -> (kh0 on partitions 0..63, kh1 on 64..127);
    # s=3+kw -> (kh2 on 0..63, zeros on 64..127).
    wpack = np.zeros((128, 6, 128), dtype=np.float32)
    for kw in range(KS):
        wpack[:64, kw, :] = kernels[:, :, 0, kw].T
        wpack[64:, kw, :] = kernels[:, :, 1, kw].T
        wpack[:64, 3 + kw, :] = kernels[:, :, 2, kw].T
    wpack = wpack.reshape(128, 6 * 128).astype(np_dt)

    bias = np.ascontiguousarray(biases.astype(np.float32).reshape(128, 1))

    in_maps = []
    for core in range(N_CORES):
        h0 = RPC * core
        xdup = np.empty((128, N_CHUNKS, QC, W), dtype=np_dt)
        for c in range(N_CHUNKS):
            q0 = h0 + 2 * PAIRS_PER_CHUNK * c
            xdup[:64, c] = xp[:, q0 : q0 + QC]
            xdup[64:, c] = xp[:, q0 + 1 : q0 + 1 + QC]
        in_maps.append(
            {
                "xdup": xdup.reshape(128, N_CHUNKS * QC * W),
                "wpack": wpack,
                "bias": bias,
            }
        )
    return in_maps


def kernel(x, kernels, biases):
    global LAST_RESULTS
    x = np.asarray(x, dtype=np.float32)
    kernels = np.asarray(kernels, dtype=np.float32)
    biases = np.asarray(biases, dtype=np.float32)

    nc = _get_program(MM_DT)
    in_maps = _prep_inputs(x, kernels, biases, MM_DT)
    res = run_bass_kernel_spmd(nc, in_maps, core_ids=list(range(N_CORES)), trace=TRACE)
    LAST_RESULTS = res

    out = np.empty((OUT_C, N_CORES * RPC, OW), dtype=np.float32)
    for c in range(N_CORES):
        out[:, RPC * c : RPC * (c + 1), :] = res.results[c]["out"].reshape(
            OUT_C, RPC, OW
        )
    return np.ascontiguousarray(out[:, :OH, :])
